# revision 9
# baseline (speedup 1.0000x reference)
"""CLIP loss kernel for trn2, 8 NeuronCores, data-parallel over the batch dim.

Strategy (v4 — no collective):
  Host prep (numpy, f64): l2-normalize both modalities (eps=1e-3 like
  F.normalize), scale by 16 (fp8 subnormal guard), transpose to
  [D, N] feature-major, cast fp8e4. spec^T is REPLICATED to all 8 cores;
  img^T is sharded by rows (each core gets its 1024-row slice's columns).
  This removes the on-device normalize/transpose preamble AND the
  AllGather + first-collective barrier of the v1 design (~85us).

  Device (SPMD, per core c):
  1. DMA in imgT [512, 1024] fp8 + specT [512, 8192] fp8, finest chunks
     first so the first matmul can start ~10us in; dummy matmuls on a
     scratch tile warm the PE clock (HAM) during the DMA ramp.
  2. main loop over 4 column groups x 8 row tiles: logits block [128, 2048]
     = imgT.T @ specT via fp8 DoubleRow matmuls (K=256 per MM), PSUM f32.
  3. ACT Exp (scale = s/256 as per-partition AP) -> bf16 e tile; DVE
     tensor_tensor_reduce fuses racc += e with a free-dim reduce of the
     updated racc slice -> cumulative row sums per (g, m); host
     differencing recovers per-block row sums. m==0 writes exp output
     directly into racc with ACT accum_out providing the base cumsum.
  4. racc shipped out per group (overlaps next group); rowcum [128, 32]
     shipped at the end.
  Host: col sums from racc, row sums from rowcum diffs, diag in f64 from
  the f32 inputs; logs and means -> scalar loss.
"""

import os
from contextlib import ExitStack

import numpy as np

import concourse.bass as bass
import concourse.mybir as mybir
from concourse import bacc, tile
from concourse.bass_utils import run_bass_kernel_spmd

N, D, C = 8192, 512, 8
NL = N // C  # 1024 local rows per core
P = 128
T = NL // P  # 8 row tiles per core
KC = D // P  # 4 contraction chunks of 128
G = 4        # column groups
GW = N // G  # 2048 columns per group

f32 = mybir.dt.float32
bf16 = mybir.dt.bfloat16
fp8 = mybir.dt.float8e4
FA = mybir.ActivationFunctionType
ALU = mybir.AluOpType

NORM_EPS = 1e-3
# fp8 operands pre-scaled by 16 to stay out of the subnormal range; the
# matmul result is 256x too big, compensated in the exp scale.
FP8_PRESCALE = 16.0

_cache: dict = {}


def _build(scale: float):
    nc = bacc.Bacc("TRN2", target_bir_lowering=False, debug=False)
    imgT = nc.dram_tensor("imgT", [D, NL], fp8, kind="ExternalInput")
    specT = nc.dram_tensor("specT", [D, N], fp8, kind="ExternalInput")
    racc_o = nc.dram_tensor("racc_o", [P, N], bf16, kind="ExternalOutput")
    rowcum_o = nc.dram_tensor("rowcum_o", [P, G * T], f32, kind="ExternalOutput")

    exp_scale = scale / (FP8_PRESCALE * FP8_PRESCALE)

    with tile.TileContext(nc) as tc, ExitStack() as ctx:
        const = ctx.enter_context(tc.tile_pool(name="const", bufs=1))
        pers = ctx.enter_context(tc.tile_pool(name="pers", bufs=1))
        ps = ctx.enter_context(tc.tile_pool(name="ps", bufs=2, space="PSUM"))
        ep = ctx.enter_context(tc.tile_pool(name="e", bufs=4))

        # preload the exp activation table before anything else needs ACT
        warm = const.tile([P, 1], f32, name="actwarm")
        nc.vector.memset(warm, 1.0)
        nc.scalar.activation(warm, warm, FA.Exp)
        scl = const.tile([P, 1], f32, name="expscale")
        nc.vector.memset(scl, exp_scale)
        wsrc = const.tile([P, 512], fp8, name="warmsrc")
        nc.vector.memset(wsrc, 0.25)

        iT = pers.tile([P, KC, NL], fp8, name="iT")
        sT = pers.tile([P, KC, N], fp8, name="sT")
        racc = pers.tile([P, N], bf16, name="racc")
        rowcum = pers.tile([P, G * T], f32, name="rowcum")

        # input DMAs ordered by first-use: the first MM (q=0) contracts
        # k-chunks 0,1 of img tile 0 and spec cols 0:512; q=1 needs 2,3
        for k in (0, 1):
            nc.sync.dma_start(iT[:, k, 0:P], imgT.ap()[k * P : (k + 1) * P, 0:P])
        for k in (0, 1):
            nc.sync.dma_start(sT[:, k, 0:512], specT.ap()[k * P : (k + 1) * P, 0:512])
        for k in (0, 1):
            nc.sync.dma_start(iT[:, k, P:NL], imgT.ap()[k * P : (k + 1) * P, P:NL])
        for k in (0, 1):
            nc.sync.dma_start(
                sT[:, k, 512:GW], specT.ap()[k * P : (k + 1) * P, 512:GW]
            )
        for k in (2, 3):
            nc.sync.dma_start(iT[:, k, :], imgT.ap()[k * P : (k + 1) * P, :])
        for k in (2, 3):
            nc.sync.dma_start(sT[:, k, 0:GW], specT.ap()[k * P : (k + 1) * P, 0:GW])
        for g in range(1, G):
            cs = slice(GW * g, GW * (g + 1))
            for k in range(KC):
                nc.sync.dma_start(
                    sT[:, k, cs], specT.ap()[k * P : (k + 1) * P, cs]
                )

        if os.environ.get("KERNEL_NO_WARMMM") != "1":
            # HAM pre-warm: ~3us of dummy matmuls on the PE while DMAs land
            wps = ps.tile([P, GW], f32, tag="mm")
            for _ in range(7):
                nc.tensor.matmul(
                    wps[:, 0:512], wsrc[:, 0:P], wsrc, start=True, stop=True
                )

        with nc.allow_low_precision("bf16 exp-sum accumulation, error ~0.5% -> <1e-3 on loss"):
            for g in range(G):
                gsl = racc[:, GW * g : GW * (g + 1)]
                for m in range(T):
                    pm = ps.tile([P, GW], f32, tag="mm")
                    # fp8 DoubleRow: each matmul contracts 2 k-chunks (K=256)
                    for q in range(KC // 2):
                        for ns in range(GW // 512):
                            cs = slice(GW * g + 512 * ns, GW * g + 512 * (ns + 1))
                            nc.tensor.matmul(
                                pm[:, 512 * ns : 512 * (ns + 1)],
                                iT[:, 2 * q : 2 * q + 2, P * m : P * (m + 1)],
                                sT[:, 2 * q : 2 * q + 2, cs],
                                start=(q == 0),
                                stop=(q == KC // 2 - 1),
                                perf_mode=mybir.MatmulPerfMode.DoubleRow,
                            )
                    idx = g * T + m
                    if m == 0:
                        # first row tile: exp lands directly in racc; ACT
                        # accumulator provides the cumsum base
                        nc.scalar.activation(
                            gsl, pm, FA.Exp, scale=scl,
                            accum_out=rowcum[:, idx : idx + 1],
                        )
                    elif os.environ.get("KERNEL_NO_TTR") == "1":
                        e = ep.tile([P, GW], bf16, tag="e")
                        nc.scalar.activation(
                            e, pm, FA.Exp, scale=scl,
                            accum_out=rowcum[:, idx : idx + 1],
                        )
                        nc.vector.tensor_add(out=gsl, in0=gsl, in1=e)
                    else:
                        e = ep.tile([P, GW], bf16, tag="e")
                        nc.scalar.activation(e, pm, FA.Exp, scale=scl)
                        # racc += e fused with rowsum of the updated racc
                        nc.vector.tensor_tensor_reduce(
                            out=gsl,
                            in0=gsl,
                            in1=e,
                            scale=1.0,
                            scalar=0.0,
                            op0=ALU.add,
                            op1=ALU.add,
                            accum_out=rowcum[:, idx : idx + 1],
                        )
                # racc[g] complete: ship it out now, overlapping next g
                nc.sync.dma_start(racc_o.ap()[:, GW * g : GW * (g + 1)], gsl)

        nc.sync.dma_start(rowcum_o.ap(), rowcum)

    nc.compile()
    return nc


def _ensure_ntff_hook():
    """antenv.axon_hooks is absent on this image; provide the tiny get/set
    registry and register trn_agent_boot's ctypes NTFF hook so trace=True
    works. Only used from test runs (KERNEL_TRACE=1)."""
    import sys
    import types

    try:
        import antenv.axon_hooks  # noqa: F401
        return
    except ImportError:
        pass
    mod = types.ModuleType("antenv.axon_hooks")
    _state = {"hook": None}
    mod.set_axon_ntff_profile_hook = lambda h: _state.__setitem__("hook", h)
    mod.get_axon_ntff_profile_hook = lambda: _state["hook"]
    import antenv

    sys.modules["antenv.axon_hooks"] = mod
    antenv.axon_hooks = mod
    try:
        from trn_agent_boot.trn_boot import _ntff_profile_via_ctypes

        mod.set_axon_ntff_profile_hook(
            _ntff_profile_via_ctypes("/opt/axon/libaxon_pjrt.so")
        )
    except Exception as e:  # degrade to no tracing
        print(f"NTFF hook setup failed: {e}")


def kernel(image_features, spectrum_features, logit_scale):
    import ml_dtypes

    scale = float(np.asarray(logit_scale))
    key = (round(scale, 9),
           os.environ.get("KERNEL_NO_TTR"), os.environ.get("KERNEL_NO_WARMMM"))
    if key not in _cache:
        _cache[key] = _build(scale)
    nc = _cache[key]

    img64 = np.asarray(image_features, dtype=np.float64)
    spec64 = np.asarray(spectrum_features, dtype=np.float64)
    ni = np.maximum(np.sqrt(np.sum(img64 * img64, axis=1, keepdims=True)), NORM_EPS)
    ns = np.maximum(np.sqrt(np.sum(spec64 * spec64, axis=1, keepdims=True)), NORM_EPS)
    img_n = img64 / ni
    spec_n = spec64 / ns

    imgT_full = np.ascontiguousarray(
        (img_n.T * FP8_PRESCALE).astype(ml_dtypes.float8_e4m3)
    )  # [D, N]
    specT = np.ascontiguousarray(
        (spec_n.T * FP8_PRESCALE).astype(ml_dtypes.float8_e4m3)
    )  # [D, N]

    in_maps = [
        {"imgT": np.ascontiguousarray(imgT_full[:, c * NL : (c + 1) * NL]),
         "specT": specT}
        for c in range(C)
    ]
    trace = os.environ.get("KERNEL_TRACE") == "1"
    if trace:
        _ensure_ntff_hook()
    res = run_bass_kernel_spmd(nc, in_maps, core_ids=list(range(C)), trace=trace)
    if trace:
        print(f"HW exec time: {res.exec_time_ns} ns (mean {res.mean_exec_time_ns})")

    # [C, P, G, T] cumulative row sums within each group
    rc = np.stack([r["rowcum_o"] for r in res.results]).astype(np.float64)
    rc = rc.reshape(C, P, G, T)
    rows_e = rc.copy()
    if os.environ.get("KERNEL_NO_TTR") != "1":
        rows_e[:, :, :, 1:] = rc[:, :, :, 1:] - rc[:, :, :, :-1]
    rowsum = rows_e.sum(axis=2)  # [C, P, T]
    cs = np.stack(
        [r["racc_o"].astype(np.float64).sum(axis=0) for r in res.results]
    )  # [C, N]

    diag = scale * np.sum(img_n * spec_n, axis=1)  # [N], f64 exact
    diag_sum = float(np.sum(diag))
    lse_i_sum = float(np.sum(np.log(rowsum)))
    lse_s_sum = float(np.sum(np.log(cs.sum(axis=0))))
    loss = 0.5 * ((lse_i_sum - diag_sum) / N + (lse_s_sum - diag_sum) / N)
    return np.float32(loss)


# revision 10
# speedup vs baseline: 1.2065x; 1.2065x over previous
"""CLIP loss kernel for trn2, 8 NeuronCores, data-parallel over the batch dim.

Strategy (v5 — no collective):
  Host prep (numpy, f64): l2-normalize both modalities (eps=1e-3 like
  F.normalize), scale by 16 (fp8 subnormal guard), transpose to
  [D, N] feature-major, cast fp8e4. spec^T is REPLICATED to all 8 cores;
  img^T is sharded by rows (each core gets its 1024-row slice's columns).
  This removes the on-device normalize/transpose preamble AND the
  AllGather + first-collective barrier of the v1 design (~85us).

  Device (SPMD, per core c):
  1. DMA in imgT [512, 1024] fp8 + specT [512, 8192] fp8, ordered by
     first-use (block-0 slivers first) so the first matmul starts ~9us in.
  2. main loop over 4 column groups x 8 row tiles: logits block [128, 2048]
     = imgT.T @ specT via fp8 DoubleRow matmuls (K=256 per MM), PSUM f32,
     2 PSUM buffers ping-ponged against the ACT exp.
  3. ACT Exp (scale = s/256 as per-partition AP; an immediate scale
     measures ~15% slower, use the AP) -> bf16 e tile, with accum_out
     giving the block's row sums for free; DVE adds e into racc [128,8192]
     (column partials, stratified by partition). m==0 writes exp output
     directly into racc, skipping the add.
  4. racc shipped out per group (overlaps next group); rowsum [128, 32]
     shipped at the end.
  Host: col sums from racc, row sums direct, diag in f64 from the f32
  inputs; logs and means -> scalar loss.
"""

import os
from contextlib import ExitStack

import numpy as np

import concourse.bass as bass
import concourse.mybir as mybir
from concourse import bacc, tile
from concourse.bass_utils import run_bass_kernel_spmd

N, D, C = 8192, 512, 8
NL = N // C  # 1024 local rows per core
P = 128
T = NL // P  # 8 row tiles per core
KC = D // P  # 4 contraction chunks of 128
G = 4        # column groups
GW = N // G  # 2048 columns per group

f32 = mybir.dt.float32
bf16 = mybir.dt.bfloat16
fp8 = mybir.dt.float8e4
FA = mybir.ActivationFunctionType

NORM_EPS = 1e-3
# fp8 operands pre-scaled by 16 to stay out of the subnormal range; the
# matmul result is 256x too big, compensated in the exp scale.
FP8_PRESCALE = 16.0

_cache: dict = {}


def _build(scale: float):
    nc = bacc.Bacc("TRN2", target_bir_lowering=False, debug=False)
    imgT = nc.dram_tensor("imgT", [D, NL], fp8, kind="ExternalInput")
    specT = nc.dram_tensor("specT", [D, N], fp8, kind="ExternalInput")
    racc_o = nc.dram_tensor("racc_o", [P, N], bf16, kind="ExternalOutput")
    rowsum_o = nc.dram_tensor("rowsum_o", [P, G * T], f32, kind="ExternalOutput")

    exp_scale = scale / (FP8_PRESCALE * FP8_PRESCALE)

    with tile.TileContext(nc) as tc, ExitStack() as ctx:
        const = ctx.enter_context(tc.tile_pool(name="const", bufs=1))
        pers = ctx.enter_context(tc.tile_pool(name="pers", bufs=1))
        ps = ctx.enter_context(tc.tile_pool(name="ps", bufs=2, space="PSUM"))
        ep = ctx.enter_context(tc.tile_pool(name="e", bufs=4))

        # preload the exp activation table before anything else needs ACT
        warm = const.tile([P, 1], f32, name="actwarm")
        nc.vector.memset(warm, 1.0)
        nc.scalar.activation(warm, warm, FA.Exp)
        scl = const.tile([P, 1], f32, name="expscale")
        nc.vector.memset(scl, exp_scale)

        iT = pers.tile([P, KC, NL], fp8, name="iT")
        sT = pers.tile([P, KC, N], fp8, name="sT")
        racc = pers.tile([P, N], bf16, name="racc")
        rowsum = pers.tile([P, G * T], f32, name="rowsum")

        # input DMAs ordered by first-use. Block (g=0, m=0) consumes
        # iT[*][0:128] and sT[*][0:2048]; later m tiles consume the iT
        # remainders; later groups the sT remainders.
        for k in range(KC):
            nc.sync.dma_start(iT[:, k, 0:P], imgT.ap()[k * P : (k + 1) * P, 0:P])
        for k in (0, 1):
            nc.sync.dma_start(sT[:, k, 0:512], specT.ap()[k * P : (k + 1) * P, 0:512])
        for k in (0, 1):
            nc.sync.dma_start(
                sT[:, k, 512:GW], specT.ap()[k * P : (k + 1) * P, 512:GW]
            )
        for k in (2, 3):
            nc.sync.dma_start(sT[:, k, 0:GW], specT.ap()[k * P : (k + 1) * P, 0:GW])
        for k in range(KC):
            nc.sync.dma_start(iT[:, k, P:NL], imgT.ap()[k * P : (k + 1) * P, P:NL])
        for g in range(1, G):
            cs = slice(GW * g, GW * (g + 1))
            for k in range(KC):
                nc.sync.dma_start(
                    sT[:, k, cs], specT.ap()[k * P : (k + 1) * P, cs]
                )

        with nc.allow_low_precision("bf16 exp-sum accumulation, error ~0.5% -> <1e-3 on loss"):
            for g in range(G):
                gsl = racc[:, GW * g : GW * (g + 1)]
                for m in range(T):
                    pm = ps.tile([P, GW], f32, tag="mm")
                    # fp8 DoubleRow: each matmul contracts 2 k-chunks (K=256)
                    for q in range(KC // 2):
                        for ns in range(GW // 512):
                            cs = slice(GW * g + 512 * ns, GW * g + 512 * (ns + 1))
                            nc.tensor.matmul(
                                pm[:, 512 * ns : 512 * (ns + 1)],
                                iT[:, 2 * q : 2 * q + 2, P * m : P * (m + 1)],
                                sT[:, 2 * q : 2 * q + 2, cs],
                                start=(q == 0),
                                stop=(q == KC // 2 - 1),
                                perf_mode=mybir.MatmulPerfMode.DoubleRow,
                            )
                    idx = g * T + m
                    if m == 0:
                        # first row tile: exp lands directly in racc
                        nc.scalar.activation(
                            gsl, pm, FA.Exp, scale=scl,
                            accum_out=rowsum[:, idx : idx + 1],
                        )
                    else:
                        e = ep.tile([P, GW], bf16, tag="e")
                        nc.scalar.activation(
                            e, pm, FA.Exp, scale=scl,
                            accum_out=rowsum[:, idx : idx + 1],
                        )
                        nc.vector.tensor_add(out=gsl, in0=gsl, in1=e)
                # racc[g] complete: ship it out now, overlapping next g
                nc.sync.dma_start(racc_o.ap()[:, GW * g : GW * (g + 1)], gsl)

        nc.sync.dma_start(rowsum_o.ap(), rowsum)

    nc.compile()
    return nc


def _ensure_ntff_hook():
    """antenv.axon_hooks is absent on this image; provide the tiny get/set
    registry and register trn_agent_boot's ctypes NTFF hook so trace=True
    works. Only used from test runs (KERNEL_TRACE=1)."""
    import sys
    import types

    try:
        import antenv.axon_hooks  # noqa: F401
        return
    except ImportError:
        pass
    mod = types.ModuleType("antenv.axon_hooks")
    _state = {"hook": None}
    mod.set_axon_ntff_profile_hook = lambda h: _state.__setitem__("hook", h)
    mod.get_axon_ntff_profile_hook = lambda: _state["hook"]
    import antenv

    sys.modules["antenv.axon_hooks"] = mod
    antenv.axon_hooks = mod
    try:
        from trn_agent_boot.trn_boot import _ntff_profile_via_ctypes

        mod.set_axon_ntff_profile_hook(
            _ntff_profile_via_ctypes("/opt/axon/libaxon_pjrt.so")
        )
    except Exception as e:  # degrade to no tracing
        print(f"NTFF hook setup failed: {e}")


def kernel(image_features, spectrum_features, logit_scale):
    import ml_dtypes

    scale = float(np.asarray(logit_scale))
    key = round(scale, 9)
    if key not in _cache:
        _cache[key] = _build(scale)
    nc = _cache[key]

    img64 = np.asarray(image_features, dtype=np.float64)
    spec64 = np.asarray(spectrum_features, dtype=np.float64)
    ni = np.maximum(np.sqrt(np.sum(img64 * img64, axis=1, keepdims=True)), NORM_EPS)
    ns = np.maximum(np.sqrt(np.sum(spec64 * spec64, axis=1, keepdims=True)), NORM_EPS)
    img_n = img64 / ni
    spec_n = spec64 / ns

    imgT_full = np.ascontiguousarray(
        (img_n.T * FP8_PRESCALE).astype(ml_dtypes.float8_e4m3)
    )  # [D, N]
    specT = np.ascontiguousarray(
        (spec_n.T * FP8_PRESCALE).astype(ml_dtypes.float8_e4m3)
    )  # [D, N]

    in_maps = [
        {"imgT": np.ascontiguousarray(imgT_full[:, c * NL : (c + 1) * NL]),
         "specT": specT}
        for c in range(C)
    ]
    trace = os.environ.get("KERNEL_TRACE") == "1"
    if trace:
        _ensure_ntff_hook()
    res = run_bass_kernel_spmd(nc, in_maps, core_ids=list(range(C)), trace=trace)
    if trace:
        print(f"HW exec time: {res.exec_time_ns} ns (mean {res.mean_exec_time_ns})")

    # [C, P, T(within-group row-tile)] per-block row sums, summed over groups
    rs = np.stack([r["rowsum_o"] for r in res.results]).astype(np.float64)
    rowsum = rs.reshape(C, P, G, T).sum(axis=2)  # [C, P, T]
    cs = np.stack(
        [r["racc_o"].astype(np.float64).sum(axis=0) for r in res.results]
    )  # [C, N]

    diag = scale * np.sum(img_n * spec_n, axis=1)  # [N], f64 exact
    diag_sum = float(np.sum(diag))
    lse_i_sum = float(np.sum(np.log(rowsum)))
    lse_s_sum = float(np.sum(np.log(cs.sum(axis=0))))
    loss = 0.5 * ((lse_i_sum - diag_sum) / N + (lse_s_sum - diag_sum) / N)
    return np.float32(loss)
